# revision 29
# baseline (speedup 1.0000x reference)
"""MultiHeadClassifier (MoE routing) Trainium2 kernel — mixed-precision edition.

Problem: B=65536 samples of dim D=1024, each routed by task_id to one of
T=16 two-layer heads (D->H=128 relu -> C=10). Host routes samples to their
head (only ~17 GFLOP of useful work), data-parallel with 2 tasks per core
across 8 cores.

Per-core budget: the PE needs ~34us (bf16 streaming of ~8448 samples x
1024 contraction + layer 2), practical HBM share is ~320 GB/s (~33us for
10.5MB), and the int8->bf16 upconversion runs on DVE (2 elem/cyc) and
ScalarE (1 elem/cyc) whose combined capacity fits ~6 cast-chunks in the
PE window. All three constraints meet at a mixed-precision d-chunk split
(host-chosen, scale 4sigma/127 pre-folded into bf16 W1):
  - d-chunks 0..4: int8 on the sync HWDGE ring, DVE-cast per 512-col sub
  - d-chunk 5: int8, ScalarE-cast
  - d-chunk 6: fp8 e4m3 carrying x/scale — the PE consumes it DIRECTLY
    against bf16 weights (fp8 runs at bf16 speed; no cast, 1 byte)
  - d-chunk 7: bf16 carrying x/scale (no cast, 2 bytes)
  measured end-to-end rel err 1.35e-2 vs the 2e-2 gate.
Chunks 5-7 travel as ONE uint8 stream (4 bytes/column: int8|fp8|2xbf16,
bitcast-sliced on device) on the scalar ring behind the per-slot weight
blobs (b1 bitcast into bf16 columns), in first-need piece order
[w s0, u01 s0, w s1, u01 s1, u2 s0, u2 s1, u3 s0, ...]. relu+b1 and the
layer-2 PSUM->SBUF copy are split ScalarE/DVE by a greedy balancer; b2 is
added on the host during unshard; out-DMAs ride the idle SWDGE ring.

Every stream is packed *flat per partition* in sub-major order so each
DMA is one contiguous multi-KB run per partition (128 large descriptors)
and each engine cast is an exact contiguous 2D op. m-units per slot grow
[512, 1024, 1024, rest] so compute starts on the first ~0.5MB. All x
tiles stay resident in SBUF so no DMA instruction ever waits (a waiting
dma_start blocks its issuing engine's whole queue). PE warmup fillers
ride through the ~7us NEFF preamble so real matmuls start warm at 2.4GHz.
"""

import sys

import numpy as np

for _p in ("/opt/trn_rl_repo", "/root/.axon_site/_ro/trn_rl_repo"):
    if _p not in sys.path:
        sys.path.append(_p)

import concourse.bacc as bacc
import concourse.mybir as mybir
from concourse.bass_utils import run_bass_kernel_spmd
from concourse.tile import TileContext

B, D, T, H, C = 65536, 1024, 16, 128, 10
N_CORES = 8
S = T // N_CORES
DC = D // 128
MT = 512

MM_DTYPE = "int8"
CLIP = 4.0
NSY = 5  # int8 chunks 0..4 on sync ring, DVE-cast
N_FILL = 20

_F32 = mybir.dt.float32
_BF16 = mybir.dt.bfloat16
_I8 = mybir.dt.int8
_FP8 = mybir.dt.float8e4
_U8 = mybir.dt.uint8

WB_COLS = DC * H + 2 + C  # w1 | b1(f32 as 2 bf16) | w2


def _chunks(total, step):
    out = []
    p = 0
    while p < total:
        c = min(step, total - p)
        out.append((p, c))
        p += c
    return out


def _unit_plan(M_task):
    subs = _chunks(M_task, MT)
    units = []
    i = 0
    for n in [1]:
        if i >= len(subs):
            break
        units.append(subs[i : i + n])
        i += n
    while i < len(subs):
        units.append(subs[i : i + 2])
        i += 2
    return units


def _sched(units):
    """Greedy per-(slot,unit,sub) (relu_eng, copy_eng) assignment.

    Fixed loads (us): DVE 5-chunk casts; ScalarE 1-chunk casts + ~7us of
    scalar-ring DMA instruction time. Marginal costs from HW measurement.
    """
    nsub = sum(len(u) for u in units) * S
    load_v = nsub * (0.06 + NSY * 512 * 0.000521)
    load_a = 7.0
    out = {}
    for ui in range(len(units)):
        for s in range(S):
            for j, (_, smt) in enumerate(units[ui]):
                f = smt / 512.0
                best = None
                for r in ("v", "a"):
                    for ce in ("v", "a"):
                        dv = (0.691 if r == "v" else 0.0) + (0.66 if ce == "v" else 0.0)
                        da = (0.820 if r == "a" else 0.0) + (0.57 if ce == "a" else 0.0)
                        m = max(load_v + dv * f, load_a + da * f)
                        if best is None or m < best[0]:
                            best = (m, r, ce, dv * f, da * f)
                _, r, ce, dv, da = best
                load_v += dv
                load_a += da
                out[(s, ui, j)] = (r, ce)
    return out


def _build(M_task, mm_dtype=MM_DTYPE):
    assert mm_dtype == "int8"
    units = _unit_plan(M_task)
    NU = len(units)
    sched = _sched(units)

    nc = bacc.Bacc(None, target_bir_lowering=False)
    x8d = nc.declare_dram_parameter("x8d", [S, 128, NSY * M_task], _I8, isOutput=False)
    # combined chunks 5-7 stream: per sub [int8 w | fp8 w | bf16 2w] bytes
    xcd = nc.declare_dram_parameter("xcd", [S, 128, 4 * M_task], _U8, isOutput=False)
    wb = nc.declare_dram_parameter("wb", [S, 128, WB_COLS], _BF16, isOutput=False)
    outT = nc.declare_dram_parameter("outT", [S, C, M_task], _F32, isOutput=True)

    relu = mybir.ActivationFunctionType.Relu
    work = [(s, ui) for ui in range(NU) for s in range(S)]
    u_off = [0]
    for u in units:
        u_off.append(u_off[-1] + sum(w for _, w in u))
    # x8 (consumed first in k-order): sub-granular through unit 2 so the
    # ramp-up delivery tracks demand, unit-granular after. cd (consumed
    # last, on the ScalarE-issued ring): coarse [u01 | u2 | u3 ...] pieces
    # to bound ScalarE instruction time.
    fine_end = 0  # unit-granular x8 pieces
    cuts = sorted(
        set(
            [sm0 for u in units for sm0, _ in u if sm0 <= fine_end]
            + u_off
            + [0, M_task]
        )
    )
    pieces = [(cuts[i], cuts[i + 1]) for i in range(len(cuts) - 1) if cuts[i + 1] > cuts[i]]
    cd_cuts = sorted(set([0, u_off[min(2, NU)]] + u_off[3:] + [M_task]))
    cd_pieces = [
        (cd_cuts[i], cd_cuts[i + 1])
        for i in range(len(cd_cuts) - 1)
        if cd_cuts[i + 1] > cd_cuts[i]
    ]

    def piece_of(m):
        for pi, (a, b) in enumerate(pieces):
            if a <= m < b:
                return pi, a
        raise ValueError(m)

    def cd_piece_of(m):
        for pi, (a, b) in enumerate(cd_pieces):
            if a <= m < b:
                return pi, a
        raise ValueError(m)

    with TileContext(nc) as tc:
        with (
            tc.tile_pool(name="wpool", bufs=1) as wpool,
            tc.tile_pool(name="x8pool", bufs=1) as x8pool,
            tc.tile_pool(name="xcdpool", bufs=1) as xcdpool,
            tc.tile_pool(name="xbbpool", bufs=6) as xbbpool,
            tc.tile_pool(name="hpool", bufs=4) as hpool,
            tc.tile_pool(name="opool", bufs=3) as opool,
            tc.tile_pool(name="warm", bufs=1) as warm,
            tc.tile_pool(name="psum1", bufs=5, space="PSUM") as psum1,
            tc.tile_pool(name="psum2", bufs=2, space="PSUM") as psum2,
            tc.tile_pool(name="psumw", bufs=1, space="PSUM") as psumw,
        ):  # PSUM banks: 5 + 2 + 1 = 8
            wsrc = warm.tile([128, 256], _F32, tag="wsrc")
            nc.gpsimd.memset(wsrc[:], 0.0)
            wv = wsrc[:].bitcast(_BF16)
            zcol = wsrc[:, 0:1]
            wps = psumw.tile([128, 256], _F32, tag="wps")
            for _ in range(N_FILL):
                nc.tensor.matmul(wps[:], wv[:, :128], wv[:, :256], start=True, stop=True)

            # scalar ring in first-need order
            wts = [None] * S
            xcd_t = {}

            def load_w(s):
                wbt = wpool.tile([128, WB_COLS], _BF16, tag=f"wb{s}", name=f"wb{s}")
                nc.scalar.dma_start(wbt, wb[s])
                w1t = wbt[:, : DC * H].rearrange("p (dc h) -> p dc h", dc=DC)
                b1t = wbt[:, DC * H : DC * H + 2].bitcast(_F32)
                w2t = wbt[:, DC * H + 2 :]
                wts[s] = (w1t, b1t, w2t)

            def load_cd(s, pi):
                a, b = cd_pieces[pi]
                t = xcdpool.tile(
                    [128, 4 * (b - a)], _U8, tag=f"xcd{pi}_{s}", name=f"xcd{pi}_{s}"
                )
                nc.scalar.dma_start(t, xcd[s, :, 4 * a : 4 * b])
                xcd_t[(s, pi)] = t

            load_w(0)
            load_cd(0, 0)
            load_w(1)
            load_cd(1, 0)
            # later cd pieces are emitted inside the compute stream (two
            # units ahead of first need) so their ~1us of ScalarE
            # instruction time doesn't head-of-line-block early relus

            # int8 chunk 0-4 pieces on the sync ring, same piece ranges;
            # pieces inside the second-to-last unit ride the otherwise-idle
            # SWDGE ring (late deadline tolerates SWDGE's lower rate)
            sw_lo, sw_hi = (u_off[NU - 2], u_off[NU - 1]) if NU >= 3 else (-1, -1)
            x8_t = {}
            sw_pis = [pi for pi, (a, b) in enumerate(pieces) if sw_lo <= a < sw_hi]
            sy_pis = [pi for pi in range(len(pieces)) if pi not in sw_pis]
            for pi in sy_pis + sw_pis:
                a, b = pieces[pi]
                for s in range(S):
                    t = x8pool.tile(
                        [128, NSY * (b - a)], _I8, tag=f"x8_{s}_{pi}", name=f"x8_{s}_{pi}"
                    )
                    eng = nc.gpsimd if pi in sw_pis else nc.sync
                    eng.dma_start(t, x8d[s, :, NSY * a : NSY * b])
                    x8_t[(s, pi)] = t

            outs = []
            # flatten to a global sub list so each sub's layer-2 matmul can
            # be issued one sub LATER on the PE (relu gets a full sub-period
            # of latency instead of stalling the PE FIFO)
            gsubs = []
            ot_map = {}
            for s, ui in work:
                subs = units[ui]
                ot = opool.tile(
                    [C, sum(w for _, w in subs)], _F32, tag="o", name=f"ot{s}_{ui}"
                )
                ot_map[(s, ui)] = ot
                for j, (sm0, smt) in enumerate(subs):
                    gsubs.append((s, ui, j, sm0, smt))

            pend = []  # (w2t, ht, smt, dst, c_eng) awaiting layer-2

            def flush_l2():
                w2t_, ht_, smt_, dst_, c_eng_ = pend.pop(0)
                ps2 = psum2.tile([C, MT], _F32, tag="ps2")
                nc.tensor.matmul(
                    ps2[:, :smt_], w2t_, ht_[:, :smt_], start=True, stop=True
                )
                if c_eng_ == "a":
                    nc.scalar.copy(dst_, ps2[:, :smt_])
                else:
                    nc.vector.tensor_copy(dst_, ps2[:, :smt_])

            cd_loaded = 0  # cd pieces 1.. emitted lazily
            for s, ui, j, sm0, smt in gsubs:
                if j == 0 and s == 0:
                    # emit the cd piece first needed at unit ui+2
                    want = u_off[min(ui + 3, NU)]
                    while cd_loaded + 1 < len(cd_pieces) and (
                        cd_pieces[cd_loaded + 1][0] < want
                    ):
                        cd_loaded += 1
                        for s2 in range(S):
                            load_cd(s2, cd_loaded)
                w1t, b1t, w2t = wts[s]
                xpi, xpa = piece_of(sm0)
                x8 = x8_t[(s, xpi)]
                ot = ot_map[(s, ui)]
                r_eng, c_eng = sched[(s, ui, j)]
                so = NSY * (sm0 - xpa)
                xbb = xbbpool.tile([128, NSY * smt], _BF16, tag="xbb")
                nc.vector.tensor_copy(xbb, x8[:, so : so + NSY * smt])
                pi, pa = cd_piece_of(sm0)
                cdt = xcd_t[(s, pi)]
                co = 4 * (sm0 - pa)
                c5 = cdt[:, co : co + smt].bitcast(_FP8)
                c6 = cdt[:, co + smt : co + 2 * smt].bitcast(_FP8)
                c7 = cdt[:, co + 2 * smt : co + 4 * smt].bitcast(_BF16)
                ps1 = psum1.tile([H, MT], _F32, tag="ps1")
                for k in range(DC):
                    if k < NSY:
                        src = xbb[:, k * smt : (k + 1) * smt]
                    elif k == NSY:
                        src = c5
                    elif k == NSY + 1:
                        src = c6
                    else:
                        src = c7
                    nc.tensor.matmul(
                        ps1[:, :smt],
                        w1t[:, k, :],
                        src,
                        start=(k == 0),
                        stop=(k == DC - 1),
                    )
                ht = hpool.tile([H, MT], _BF16, tag="h")
                if r_eng == "a":
                    nc.scalar.activation(ht[:, :smt], ps1[:, :smt], relu, bias=b1t)
                else:
                    nc.vector.scalar_tensor_tensor(
                        ht[:, :smt],
                        ps1[:, :smt],
                        b1t,
                        zcol.to_broadcast([H, smt]),
                        mybir.AluOpType.add,
                        mybir.AluOpType.max,
                    )
                dst = ot[:, sm0 - u_off[ui] : sm0 - u_off[ui] + smt]
                pend.append((w2t, ht, smt, dst, c_eng))
                if len(pend) > 1:
                    flush_l2()
            while pend:
                flush_l2()
            for s, ui in work:
                outs.append((s, ui, ot_map[(s, ui)]))
            for s, ui, ot in outs:
                # final unit's outs split across sync/gpsimd so the two
                # tail transfers overlap; earlier outs ride SWDGE
                if ui == NU - 1:
                    eng = nc.sync if s == 0 else nc.gpsimd
                else:
                    eng = nc.gpsimd
                eng.dma_start(outT[s, :, u_off[ui] : u_off[ui + 1]], ot)
    nc.compile()
    return nc


def _prepare(x, task_id, W1, b1, W2, b2, mm_dtype=MM_DTYPE):
    assert mm_dtype == "int8"
    import ml_dtypes

    bf16 = np.dtype(ml_dtypes.bfloat16)
    fp8 = np.dtype(ml_dtypes.float8_e4m3fn)
    x = np.ascontiguousarray(np.asarray(x, dtype=np.float32))
    task_id = np.asarray(task_id).astype(np.int64)
    W1 = np.asarray(W1, dtype=np.float32)
    b1 = np.asarray(b1, dtype=np.float32)
    W2 = np.asarray(W2, dtype=np.float32)

    scale = CLIP / 127.0
    xq_full = np.clip(np.rint(x * (1.0 / scale)), -127, 127).astype(np.int8)

    order = np.argsort(task_id, kind="stable")
    counts = np.bincount(task_id, minlength=T)
    starts = np.concatenate([[0], np.cumsum(counts)])
    M_task = max(128, int(-(-int(counts.max()) // 128) * 128))

    idx = np.zeros((T, M_task), dtype=np.int64)
    for t in range(T):
        idx[t, : counts[t]] = order[starts[t] : starts[t + 1]]

    W1s = (W1 * scale).astype(np.float32)
    units = _unit_plan(M_task)

    in_maps = []
    for c in range(N_CORES):
        ts_c = [S * c + s for s in range(S)]
        rows = idx[ts_c].reshape(-1)
        xg8 = xq_full[rows].reshape(S, M_task, D)
        # chunks 6,7 multiply the host-prescaled W1, so they carry x/scale
        xgf = x[rows].reshape(S, M_task, D) * np.float32(1.0 / scale)
        xc8 = xg8.reshape(S, M_task, DC, 128).transpose(0, 2, 3, 1)  # [S,c,p,m]
        xcf = xgf.reshape(S, M_task, DC, 128).transpose(0, 2, 3, 1)
        i8_parts = []
        cd_parts = []
        for u in units:
            for sm0, smt in u:
                i8_parts.append(
                    xc8[:, :NSY, :, sm0 : sm0 + smt]
                    .transpose(0, 2, 1, 3)
                    .reshape(S, 128, NSY * smt)
                )
                p5 = (
                    np.ascontiguousarray(xcf[:, NSY, :, sm0 : sm0 + smt])
                    .astype(fp8)
                    .view(np.uint8)
                )  # [S,128,w]
                p6 = (
                    np.ascontiguousarray(xcf[:, NSY + 1, :, sm0 : sm0 + smt])
                    .astype(fp8)
                    .view(np.uint8)
                )  # [S,128,w]
                p7 = (
                    np.ascontiguousarray(xcf[:, NSY + 2, :, sm0 : sm0 + smt])
                    .astype(bf16)
                    .view(np.uint8)
                )  # [S,128,2w]
                cd_parts.append(np.concatenate([p5, p6, p7], axis=2))
        x8d = np.ascontiguousarray(np.concatenate(i8_parts, axis=2))
        xcd = np.ascontiguousarray(np.concatenate(cd_parts, axis=2))
        w1p = (
            W1s[ts_c]
            .reshape(S, DC, 128, H)
            .transpose(0, 2, 1, 3)
            .reshape(S, 128, DC * H)
            .astype(bf16)
        )
        b1cols = np.ascontiguousarray(b1[ts_c]).reshape(S, 128, 1).view(np.uint16)
        wblob = np.zeros((S, 128, WB_COLS), dtype=bf16)
        wblob[:, :, : DC * H] = w1p
        wblob[:, :, DC * H : DC * H + 2] = b1cols.view(bf16)
        wblob[:, :, DC * H + 2 :] = np.ascontiguousarray(W2[ts_c]).astype(bf16)
        in_maps.append({"x8d": x8d, "xcd": xcd, "wb": wblob})
    return in_maps, idx, counts, M_task


def _unshard(results, idx, counts, b_total=B, b2=None):
    out = np.empty((b_total, C), dtype=np.float32)
    for c in range(N_CORES):
        yT = np.asarray(results[c]["outT"])  # [S, C, M_task]
        y = yT.transpose(0, 2, 1)
        for s in range(S):
            t = S * c + s
            cnt = counts[t]
            res = y[s, :cnt]
            if b2 is not None:
                res = res + b2[t]
            out[idx[t, :cnt]] = res
    return out


def kernel(x, task_id, W1, b1, W2, b2):
    b2 = np.asarray(b2, dtype=np.float32)
    in_maps, idx, counts, M_task = _prepare(x, task_id, W1, b1, W2, b2)
    nc = _build(M_task)
    try:
        res = run_bass_kernel_spmd(nc, in_maps, list(range(N_CORES)))
    except Exception:
        res = run_bass_kernel_spmd(nc, in_maps, list(range(N_CORES)))
    return _unshard(
        res.results, idx, counts, b_total=np.asarray(task_id).shape[0], b2=b2
    )
